# revision 1
# baseline (speedup 1.0000x reference)
"""GCN autoencoder kernel for 8 Trainium2 NeuronCores.

Strategy (self-contained; shapes hardcoded for the graded problem):
  - Nodes row-sharded 1250/core; edge list partitioned by dst and sorted.
  - Per core: Y1 = x_slab @ W1 (PE transposes of x + matmuls), AllGather of the
    row-padded Y1 table, dma_gather of per-edge 256B rows, segment-sum via PE
    matmuls against DVE-built one-hot*weight selection matrices (32-dst column
    strips via tile_position), relu -> H, AllGather, same aggregation again,
    then z^T = W2^T @ zpre^T, AllGather z^T.
  - Decode: out = sigmoid(z_own @ z_all^T) with float32r matmuls (N=512
    chunks), ScalarE sigmoid from PSUM, 5MB/row-tile streaming stores.
"""

from contextlib import ExitStack
from dataclasses import dataclass

import numpy as np

import concourse.bass as bass
import concourse.mybir as mybir
import concourse.tile as tile
from concourse import bacc
from concourse.bass_utils import run_bass_kernel_spmd

dt = mybir.dt


@dataclass
class Cfg:
    n_nodes: int = 10000
    n_feat: int = 512
    hid: int = 32
    code: int = 16
    n_cores: int = 8
    gs: int = 32          # dst nodes per PSUM column strip
    chunk: int = 128      # edges per matmul chunk
    pad: int = 128        # bf16 elements per gather row (256B)
    decode_dt: str = "bfloat16"
    ablate: int = 4       # 1=y1+AG, 2=+layer1, 3=+layer2/zt, 4=full
    n_queues: int = 4     # SWDGE queues for parallel gather desc-gen
    Cg: tuple = ()        # per-group chunk counts (data dependent; from prep)

    @property
    def rows(self):
        return self.n_nodes // self.n_cores

    @property
    def ng(self):  # groups per core
        return -(-self.rows // self.gs)

    @property
    def mt(self):  # 128-row m-tiles per core
        return -(-self.rows // 128)

    @property
    def nch(self):  # chunks per core
        return sum(self.Cg)

    @property
    def kch(self):  # 128-row K chunks of n_feat
        return self.n_feat // 128

    @property
    def chunk_base(self):
        b, acc = [], 0
        for c in self.Cg:
            b.append(acc)
            acc += c
        return b


def prep_edges(cfg: Cfg, src, dst, ew):
    """Sort edges by dst, shard by dst range, group into gs-dst groups each
    padded to C*chunk slots. Returns per-core (gidx int16 [128, nch*chunk/16],
    wt f32 [128, nch], dmb f32 [128, nch]) and the chosen C."""
    src = np.asarray(src).astype(np.int64)
    dst = np.asarray(dst).astype(np.int64)
    ew = np.asarray(ew).astype(np.float32)
    order = np.argsort(dst, kind="stable")
    s_s, d_s, w_s = src[order], dst[order], ew[order]

    per_core = []
    maxcnt = np.zeros(cfg.ng, np.int64)
    for c in range(cfg.n_cores):
        lo = c * cfg.rows
        m = (d_s >= lo) & (d_s < lo + cfg.rows)
        sc, dc, wc = s_s[m], d_s[m] - lo, w_s[m]
        gids = dc // cfg.gs
        counts = np.bincount(gids, minlength=cfg.ng)
        maxcnt = np.maximum(maxcnt, counts)
        per_core.append((sc, dc, wc, counts))
    # per-group chunk count, uniform across cores (program uniformity)
    cfg.Cg = tuple(int(x) for x in np.maximum(1, -(-maxcnt // cfg.chunk)))

    cbase = cfg.chunk_base
    slots = cfg.nch * cfg.chunk
    outs = []
    for sc, dc, wc, counts in per_core:
        srcpad = np.zeros(slots, np.int64)
        wpad = np.zeros(slots, np.float32)
        dmbpad = np.full(slots, -1.0, np.float32)
        pos = 0
        for g in range(cfg.ng):
            cnt = counts[g]
            base = cbase[g] * cfg.chunk
            srcpad[base : base + cnt] = sc[pos : pos + cnt]
            wpad[base : base + cnt] = wc[pos : pos + cnt]
            dmbpad[base : base + cnt] = (dc[pos : pos + cnt] - g * cfg.gs).astype(
                np.float32
            )
            pos += cnt
        gidx16 = srcpad.reshape(-1, 16).T.astype(np.int16)  # [16, slots/16]
        gidx = np.tile(gidx16, (8, 1)).copy()  # [128, slots/16]
        wt = wpad.reshape(cfg.nch, cfg.chunk).T.copy()  # [128, nch]
        dmb = dmbpad.reshape(cfg.nch, cfg.chunk).T.copy()
        outs.append((gidx, wt, dmb))
    return outs


def build_nc(cfg: Cfg):
    nc = bacc.Bacc(
        "TRN2",
        target_bir_lowering=False,
        debug=False,
        enable_asserts=False,
        num_devices=cfg.n_cores,
        num_swdge_queues=cfg.n_queues,
    )
    f32 = dt.float32
    bf16 = dt.bfloat16
    N, R, HID, CODE, PAD = cfg.n_nodes, cfg.rows, cfg.hid, cfg.code, cfg.pad
    GS, CH, NG, MT, KCH = cfg.gs, cfg.chunk, cfg.ng, cfg.mt, cfg.kch
    CG, CBASE = cfg.Cg, cfg.chunk_base
    ddt = getattr(dt, cfg.decode_dt)

    # ---- external I/O ----
    xs = nc.dram_tensor("xs", [R, cfg.n_feat], f32, kind="ExternalInput").ap()
    w1 = nc.dram_tensor("w1", [cfg.n_feat, HID], f32, kind="ExternalInput").ap()
    w2 = nc.dram_tensor("w2", [HID, CODE], f32, kind="ExternalInput").ap()
    ident_d = nc.dram_tensor("ident", [128, 128], f32, kind="ExternalInput").ap()
    iota_d = nc.dram_tensor("iota", [128, GS], f32, kind="ExternalInput").ap()
    gidx_d = nc.dram_tensor(
        "gidx", [128, cfg.nch * CH // 16], dt.int16, kind="ExternalInput"
    ).ap()
    wt_d = nc.dram_tensor("wt", [128, cfg.nch], f32, kind="ExternalInput").ap()
    dmb_d = nc.dram_tensor("dmb", [128, cfg.nch], f32, kind="ExternalInput").ap()
    out_d = nc.dram_tensor("out", [R, N], f32, kind="ExternalOutput").ap()

    # ---- internal DRAM ----
    y1_own = nc.dram_tensor("y1_own", [R, PAD], bf16).ap()
    y1_all = nc.dram_tensor("y1_all", [N, PAD], bf16, addr_space="Shared").ap()
    h_own = nc.dram_tensor("h_own", [R, PAD], bf16).ap()
    h_all = nc.dram_tensor("h_all", [N, PAD], bf16, addr_space="Shared").ap()
    zt_own = nc.dram_tensor("zt_own", [CODE, R], ddt).ap()
    zt_all = nc.dram_tensor(
        "zt_all", [cfg.n_cores, CODE, R], ddt, addr_space="Shared"
    ).ap()

    groups_all = [list(range(cfg.n_cores))]

    def rows_of(m):  # valid rows in m-tile m
        return min(128, R - m * 128)

    def jmax_of(m):  # column strips in m-tile m
        return min(4, NG - 4 * m)

    # decode N-chunking: 512-wide chunks grouped 4 per PSUM tile
    nchunks = []
    n0 = 0
    while n0 < N:
        nn = min(512, N - n0)
        nchunks.append((n0, nn))
        n0 += nn
    bank_groups = [nchunks[i : i + 4] for i in range(0, len(nchunks), 4)]

    # gather call split: whole m-tiles (4 groups) per call
    GPC = 4 if NG % 4 == 0 else NG  # groups per gather call
    NCALL = NG // GPC
    GBW = max(
        CBASE[c * GPC + GPC - 1] + CG[c * GPC + GPC - 1] - CBASE[c * GPC]
        for c in range(NCALL)
    )  # widest call, in chunks

    with tile.TileContext(nc) as tc, ExitStack() as ctx:
        # ---- long-lived pools ----
        cpool = ctx.enter_context(tc.tile_pool(name="consts", bufs=1))
        spool = ctx.enter_context(tc.tile_pool(name="smat", bufs=1))
        zpool = ctx.enter_context(tc.tile_pool(name="zbits", bufs=1))

        # x-path constants first — they gate the Y1 critical path; edge
        # constants (gidx/wt/dmb) aren't needed until the first gather
        ident = cpool.tile([128, 128], f32)
        nc.sync.dma_start(ident[:], ident_d[:, :])
        w1s = cpool.tile([128, KCH, HID], f32)
        for k in range(KCH):
            nc.sync.dma_start(w1s[:, k, :], w1[k * 128 : (k + 1) * 128, :])
        w2s = cpool.tile([HID, CODE], f32)
        nc.sync.dma_start(w2s[:], w2[:, :])
        iota = cpool.tile([128, GS], f32)
        nc.sync.dma_start(iota[:], iota_d[:, :])
        gidx = cpool.tile([128, cfg.nch * CH // 16], dt.int16)
        nc.scalar.dma_start(gidx[:], gidx_d[:, :])
        wts = cpool.tile([128, cfg.nch], f32)
        nc.scalar.dma_start(wts[:], wt_d[:, :])
        dmbs = cpool.tile([128, cfg.nch], f32)
        nc.scalar.dma_start(dmbs[:], dmb_d[:, :])

        smat = spool.tile([128, cfg.nch, GS], bf16)  # selection matrices (reused)
        zts = zpool.tile([CODE, R], ddt)  # own z^T staging
        # decode operands replicated at 4 partition strips (row-grp rotation
        # lets LDWEIGHTS overlap in-flight matmuls)
        zts4 = zpool.tile([128, R], ddt)
        ztall4 = zpool.tile([128, N], ddt)
        zpreT = zpool.tile([HID, MT * 128], f32)

        # ================= phase A/B: x^T and Y1 =================
        with tc.tile_pool(name="xio", bufs=2) as xio, tc.tile_pool(
            name="xt", bufs=1
        ) as xtp, tc.tile_pool(name="pst", bufs=2, space="PSUM") as pst, tc.tile_pool(
            name="psy", bufs=2, space="PSUM"
        ) as psy, tc.tile_pool(name="stage", bufs=2) as stage:
            xT = xtp.tile([128, KCH, MT * 128], f32)
            for m in range(MT):
                rm = rows_of(m)
                xin = xio.tile([128, cfg.n_feat], f32)
                nc.sync.dma_start(xin[:rm, :], xs[m * 128 : m * 128 + rm, :])
                for k in range(KCH):
                    pt = pst.tile([128, 128], f32, space="PSUM")
                    nc.tensor.transpose(
                        pt[:, :rm],
                        xin[:rm, k * 128 : (k + 1) * 128],
                        ident[:rm, :rm],
                    )
                    nc.vector.tensor_copy(
                        xT[:, k, m * 128 : m * 128 + rm], pt[:, :rm]
                    )
            for m in range(MT):
                rm = rows_of(m)
                py = psy.tile([128, HID], f32, space="PSUM")
                for k in range(KCH):
                    nc.tensor.matmul(
                        py[:rm, :],
                        lhsT=xT[:, k, m * 128 : m * 128 + rm],
                        rhs=w1s[:, k, :],
                        start=(k == 0),
                        stop=(k == KCH - 1),
                    )
                st = stage.tile([128, PAD], bf16)
                nc.vector.memset(st[:, HID:PAD], 0.0)
                nc.vector.tensor_copy(st[:rm, 0:HID], py[:rm, :])
                nc.sync.dma_start(y1_own[m * 128 : m * 128 + rm, :], st[:rm, :])

        nc.gpsimd.collective_compute(
            "AllGather",
            mybir.AluOpType.bypass,
            replica_groups=groups_all,
            ins=[y1_own.opt()],
            outs=[y1_all.opt()],
        )

        # ================= SpMM layers =================
        def spmm(src_tab, build_s, emit_group_out, tag):
            with tc.tile_pool(name=f"gbuf_{tag}", bufs=5) as gpool, tc.tile_pool(
                name=f"psg_{tag}", bufs=4, space="PSUM"
            ) as psg:
                for call in range(NCALL):
                    gpc = min(GPC, NG - call * GPC)
                    c0 = CBASE[call * GPC]  # first chunk of this call
                    glast = call * GPC + gpc - 1
                    cpc = CBASE[glast] + CG[glast] - c0  # chunks this call
                    nidx = cpc * CH
                    gb = gpool.tile([128, GBW, PAD], bf16, tag="gb")
                    nc.gpsimd.dma_gather(
                        out_ap=gb[:, :cpc, :],
                        in_ap=src_tab[:, :],
                        idxs_ap=gidx[:, c0 * CH // 16 : (c0 + cpc) * CH // 16],
                        num_idxs=nidx,
                        num_idxs_reg=nidx,
                        elem_size=PAD,
                        single_packet=False,
                        queue_num=call % cfg.n_queues,
                    )
                    for gl in range(gpc):
                        g = call * GPC + gl
                        m, j = divmod(g, 4)
                        if j == 0:
                            pm = psg.tile([128, HID], f32, space="PSUM", tag="pm")
                        for t in range(CG[g]):
                            tg = CBASE[g] + t
                            s_t = smat[:, tg, :]
                            if build_s:
                                nc.vector.tensor_scalar(
                                    s_t,
                                    iota[:, :],
                                    dmbs[:, tg : tg + 1],
                                    wts[:, tg : tg + 1],
                                    op0=mybir.AluOpType.is_equal,
                                    op1=mybir.AluOpType.mult,
                                )
                            nc.tensor.matmul(
                                pm[j * GS : (j + 1) * GS, :],
                                lhsT=s_t,
                                rhs=gb[:, tg - c0, 0:HID],
                                start=(t == 0),
                                stop=(t == CG[g] - 1),
                                tile_position=(0, j * GS),
                            )
                        if j == jmax_of(m) - 1:
                            emit_group_out(m, pm)

        # ---- layer 1: H = relu(A @ Y1), padded + AllGather ----
        if cfg.ablate >= 2:
            with tc.tile_pool(name="hstage", bufs=2) as hstage:

                def l1_out(m, pm):
                    rm = rows_of(m)
                    st = hstage.tile([128, PAD], bf16)
                    nc.vector.memset(st[:, HID:PAD], 0.0)
                    nc.scalar.activation(
                        st[:rm, 0:HID],
                        pm[:rm, :],
                        mybir.ActivationFunctionType.Relu,
                    )
                    nc.sync.dma_start(
                        h_own[m * 128 : m * 128 + rm, :], st[:rm, :]
                    )

                spmm(y1_all, build_s=True, emit_group_out=l1_out, tag="l1")

            nc.gpsimd.collective_compute(
                "AllGather",
                mybir.AluOpType.bypass,
                replica_groups=groups_all,
                ins=[h_own.opt()],
                outs=[h_all.opt()],
            )

        # ---- layer 2: zpre = A @ H, transposed into zpreT ----
        if cfg.ablate >= 3:
            _layer2(tc, nc, cfg, spmm, rows_of, ident, zpreT, w2s, zts,
                    zt_own, zt_all, zts4, ztall4, h_all, groups_all)

        # ================= decode =================
        if cfg.ablate >= 4:
            _decode(tc, nc, cfg, rows_of, bank_groups, zts4, ztall4, out_d)

    nc.compile()
    return nc


def _layer2(tc, nc, cfg, spmm, rows_of, ident, zpreT, w2s, zts, zt_own,
            zt_all, zts4, ztall4, h_all, groups_all):
    f32 = dt.float32
    R, HID, CODE = cfg.rows, cfg.hid, cfg.code
    with tc.tile_pool(name="zstage", bufs=2) as zstage, tc.tile_pool(
        name="pstz", bufs=2, space="PSUM"
    ) as pstz:

        def l2_out(m, pm):
            rm = rows_of(m)
            zp = zstage.tile([128, HID], f32)
            nc.vector.tensor_copy(zp[:rm, :], pm[:rm, :])
            ptz = pstz.tile([HID, 128], f32, space="PSUM")
            nc.tensor.transpose(ptz[:, :rm], zp[:rm, :], ident[:rm, :rm])
            nc.vector.tensor_copy(
                zpreT[:, m * 128 : m * 128 + rm], ptz[:, :rm]
            )

        spmm(h_all, build_s=False, emit_group_out=l2_out, tag="l2")

        # z^T = W2^T @ zpre^T   [CODE, R]
        zn0 = 0
        while zn0 < R:
            zn = min(512, R - zn0)
            pzc = pstz.tile([CODE, 512], f32, space="PSUM", tag="pzc")
            nc.tensor.matmul(
                pzc[:, :zn],
                lhsT=w2s[:, :],
                rhs=zpreT[:, zn0 : zn0 + zn],
                start=True,
                stop=True,
            )
            nc.vector.tensor_copy(zts[:, zn0 : zn0 + zn], pzc[:, :zn])
            zn0 += zn
        nc.sync.dma_start(zt_own[:, :], zts[:, :])

    nc.gpsimd.collective_compute(
        "AllGather",
        mybir.AluOpType.bypass,
        replica_groups=groups_all,
        ins=[zt_own.opt()],
        outs=[zt_all.opt()],
    )
    # load z^T gathered into 4 partition strips: ztall4[32s+p, r*R+j]
    CODE = cfg.code
    for s in range(4):
        nc.sync.dma_start(
            ztall4[32 * s : 32 * s + CODE, :].rearrange(
                "p (r j) -> p r j", r=cfg.n_cores
            ),
            zt_all.rearrange("r p j -> p r j"),
        )
        nc.sync.dma_start(zts4[32 * s : 32 * s + CODE, :], zt_own[:, :])


def _decode(tc, nc, cfg, rows_of, bank_groups, zts4, ztall4, out_d):
    f32 = dt.float32
    N, CODE = cfg.n_nodes, cfg.code
    with tc.tile_pool(name="obuf", bufs=2) as obuf, tc.tile_pool(
        name="psd", bufs=2, space="PSUM"
    ) as psd:
        qq = 0
        for m in range(cfg.mt):
            rm = rows_of(m)
            ob = obuf.tile([128, N], f32)
            for bg in bank_groups:
                # only the last chunk of a group can be short, so the
                # written psum region [0, w) is dense
                w = sum(nn for _, nn in bg)
                pd = psd.tile([128, 2048], f32, space="PSUM")
                for q, (nn0, nn) in enumerate(bg):
                    s = qq % 4  # rotate PE row strips so LDW pipelines
                    qq += 1
                    p0 = 32 * s
                    nc.tensor.matmul(
                        pd[:rm, q * 512 : q * 512 + nn],
                        lhsT=zts4[p0 : p0 + CODE, m * 128 : m * 128 + rm],
                        rhs=ztall4[p0 : p0 + CODE, nn0 : nn0 + nn],
                        start=True,
                        stop=True,
                        tile_position=(p0, 0),
                    )
                b0 = bg[0][0]
                nc.scalar.activation(
                    ob[:rm, b0 : b0 + w],
                    pd[:rm, :w],
                    mybir.ActivationFunctionType.Sigmoid,
                )
            nc.sync.dma_start(out_d[m * 128 : m * 128 + rm, :], ob[:rm, :])


def _host_prep(cfg: Cfg, x, W1, W2, edge_weight, src, dst):
    per_core_edges = prep_edges(cfg, src, dst, edge_weight)
    ident = np.eye(128, dtype=np.float32)
    iota0 = np.tile(np.arange(cfg.gs, dtype=np.float32), (128, 1)).copy()
    in_maps = []
    x = np.ascontiguousarray(np.asarray(x, dtype=np.float32))
    W1 = np.ascontiguousarray(np.asarray(W1, dtype=np.float32))
    W2 = np.ascontiguousarray(np.asarray(W2, dtype=np.float32))
    for c in range(cfg.n_cores):
        gidx, wt, dmb = per_core_edges[c]
        in_maps.append(
            {
                "xs": np.ascontiguousarray(x[c * cfg.rows : (c + 1) * cfg.rows]),
                "w1": W1,
                "w2": W2,
                "ident": ident,
                "iota": iota0,
                "gidx": np.ascontiguousarray(gidx),
                "wt": np.ascontiguousarray(wt),
                "dmb": np.ascontiguousarray(dmb),
            }
        )
    return in_maps


def kernel(x, W1, W2, edge_weight, src, dst, trace=False):
    cfg = Cfg()
    in_maps = _host_prep(cfg, x, W1, W2, edge_weight, src, dst)
    nc = build_nc(cfg)
    res = run_bass_kernel_spmd(
        nc, in_maps, core_ids=list(range(cfg.n_cores)), trace=trace
    )
    out = np.concatenate([r["out"] for r in res.results], axis=0)
    if trace:
        kernel.last_results = res
    return np.ascontiguousarray(out.astype(np.float32))



# revision 4
# speedup vs baseline: 1.9030x; 1.9030x over previous
"""GCN autoencoder kernel for 8 Trainium2 NeuronCores — dense-block SpMM.

Strategy (self-contained; shapes hardcoded for the graded problem):
  - Nodes row-sharded 1250/core. Host precomputes, per core, the dense
    adjacency slab AB[10000, 1250] bf16 (AB[s, j] = A_hat[base+j, s], i.e.
    column j holds dst base+j's incoming edge weights) — graph-constant
    layout prep. Host also ships x pre-transposed per core.
  - Y1 = x @ W1 computed row-wise per m-tile (lhsT = xT slabs).
    AllGather of unpadded bf16 Y1 rows [10000, 32].
  - SpMM layers run transposed on PE: out^T[F, 1250] = sum_k T_k^T @ AB_k
    with the tiny table k-tile as stationary weights and the dense AB
    k-block streamed from HBM as the moving operand (no per-edge DMA
    gather, no DVE one-hot builds, no GpSimd descriptor generation).
  - relu on ScalarE; Hw = H @ W2 fused with the layout flip back to rows
    (lhsT = H^T m-slices); AllGather Hw rows; layer 2 same dense scheme
    gives z^T [16, 1250] directly; AllGather z^T.
  - Decode: out = sigmoid(z_own @ z_all^T) bf16 matmuls (N=512 chunks,
    4-strip row rotation), ScalarE sigmoid PSUM->SBUF, bf16 output rows
    (host casts back to f32).
"""

from contextlib import ExitStack
from dataclasses import dataclass

import numpy as np
import ml_dtypes

import concourse.bass as bass
import concourse.mybir as mybir
import concourse.tile as tile
from concourse import bacc
from concourse.bass_utils import run_bass_kernel_spmd

dt = mybir.dt


@dataclass
class Cfg:
    n_nodes: int = 10000
    n_feat: int = 512
    hid: int = 32
    code: int = 16
    n_cores: int = 8
    ab_bufs: int = 12     # streamed AB k-blocks in flight

    @property
    def rows(self):
        return self.n_nodes // self.n_cores

    @property
    def kt(self):  # 128-row k-tiles over the node axis (last is partial)
        return -(-self.n_nodes // 128)

    @property
    def mt(self):  # 128-row m-tiles per core
        return -(-self.rows // 128)

    @property
    def kc(self):  # 128-row chunks of n_feat
        return self.n_feat // 128


def _nchunks(total, step=512):
    out = []
    n0 = 0
    while n0 < total:
        out.append((n0, min(step, total - n0)))
        n0 += step
    return out


def build_nc(cfg: Cfg):
    nc = bacc.Bacc(
        "TRN2",
        target_bir_lowering=False,
        debug=False,
        enable_asserts=False,
        num_devices=cfg.n_cores,
    )
    f32 = dt.float32
    bf16 = dt.bfloat16
    N, R, HID, CODE = cfg.n_nodes, cfg.rows, cfg.hid, cfg.code
    KT, MT, KC = cfg.kt, cfg.mt, cfg.kc
    KLAST = N - (KT - 1) * 128  # rows in the last k-tile

    # ---- external I/O ----
    xT_d = nc.dram_tensor("xt", [cfg.n_feat, R], f32, kind="ExternalInput").ap()
    w1_d = nc.dram_tensor("w1", [cfg.n_feat, HID], f32, kind="ExternalInput").ap()
    w2_d = nc.dram_tensor("w2", [HID, CODE], f32, kind="ExternalInput").ap()
    ab_d = nc.dram_tensor("ab", [N, R], bf16, kind="ExternalInput").ap()
    out_d = nc.dram_tensor("out", [R, N], bf16, kind="ExternalOutput").ap()

    # ---- internal DRAM ----
    y1_own = nc.dram_tensor("y1_own", [R, HID], bf16).ap()
    y1_all = nc.dram_tensor("y1_all", [N, HID], bf16, addr_space="Shared").ap()
    hw_own = nc.dram_tensor("hw_own", [R, CODE], bf16).ap()
    hw_all = nc.dram_tensor("hw_all", [N, CODE], bf16, addr_space="Shared").ap()
    zt_own = nc.dram_tensor("zt_own", [CODE, R], bf16).ap()
    zt_all = nc.dram_tensor(
        "zt_all", [cfg.n_cores, CODE, R], bf16, addr_space="Shared"
    ).ap()

    groups_all = [list(range(cfg.n_cores))]
    rchunks = _nchunks(R)          # psum n-chunking over the 1250 dst cols

    def rows_of(m):
        return min(128, R - m * 128)

    def kr_of(k):
        return 128 if k < KT - 1 else KLAST

    # decode N-chunking: 512-wide chunks grouped 4 per PSUM tile
    bank_groups = []
    ncs = _nchunks(N)
    for i in range(0, len(ncs), 4):
        bank_groups.append(ncs[i : i + 4])

    with tile.TileContext(nc) as tc, ExitStack() as ctx:
        cpool = ctx.enter_context(tc.tile_pool(name="consts", bufs=1))
        zpool = ctx.enter_context(tc.tile_pool(name="zbits", bufs=1))

        w1s = cpool.tile([128, KC, HID], f32)
        for k in range(KC):
            nc.sync.dma_start(w1s[:, k, :], w1_d[k * 128 : (k + 1) * 128, :])
        w2s = cpool.tile([HID, CODE], f32)
        nc.sync.dma_start(w2s[:], w2_d[:, :])
        w2b = cpool.tile([HID, CODE], bf16)
        nc.vector.tensor_copy(w2b[:], w2s[:])

        # node-feature tables (stationary operands for the dense SpMM)
        ytab = cpool.tile([128, KT, HID], bf16)
        htab = cpool.tile([128, KT, CODE], bf16)
        # decode operands replicated at 4 partition strips
        zts4 = zpool.tile([128, R], bf16)
        ztall4 = zpool.tile([128, N], bf16)

        # ================= phase Y1: y1 = x @ W1 (row layout) ============
        with tc.tile_pool(name="xts", bufs=1) as xtp, tc.tile_pool(
            name="psy", bufs=2, space="PSUM"
        ) as psy, tc.tile_pool(name="ystage", bufs=2) as ystage:
            xTs = xtp.tile([128, KC, R], f32)
            nc.sync.dma_start(
                xTs[:, :, :], xT_d.rearrange("(k p) n -> p k n", p=128)
            )
            for m in range(MT):
                rm = rows_of(m)
                py = psy.tile([128, HID], f32, space="PSUM")
                for k in range(KC):
                    nc.tensor.matmul(
                        py[:rm, :],
                        lhsT=xTs[:, k, m * 128 : m * 128 + rm],
                        rhs=w1s[:, k, :],
                        start=(k == 0),
                        stop=(k == KC - 1),
                    )
                st = ystage.tile([128, HID], bf16)
                nc.vector.tensor_copy(st[:rm, :], py[:rm, :])
                nc.sync.dma_start(y1_own[m * 128 : m * 128 + rm, :], st[:rm, :])

        nc.gpsimd.collective_compute(
            "AllGather",
            mybir.AluOpType.bypass,
            replica_groups=groups_all,
            ins=[y1_own.opt()],
            outs=[y1_all.opt()],
        )

        # table load: [10000, F] -> [128, KT, F]
        nfull = (KT - 1) * 128
        nc.sync.dma_start(
            ytab[:, : KT - 1, :],
            y1_all[:nfull, :].rearrange("(k p) f -> p k f", p=128),
        )
        nc.sync.dma_start(ytab[:KLAST, KT - 1, :], y1_all[nfull:, :])

        # ================= dense SpMM layers =================
        def spmm_T(tab, fdim, pst, abp, tag):
            """psum[fdim, R] = sum_k tab_k^T @ AB_k  (streamed AB blocks)."""
            ps = pst.tile([fdim, R], f32, space="PSUM", tag=f"ps_{tag}")
            for k in range(KT):
                kr = kr_of(k)
                ab = abp.tile([128, R], bf16, tag="ab")
                nc.sync.dma_start(
                    ab[:kr, :], ab_d[k * 128 : k * 128 + kr, :]
                )
                for n0, nn in rchunks:
                    nc.tensor.matmul(
                        ps[:, n0 : n0 + nn],
                        lhsT=tab[:kr, k, :],
                        rhs=ab[:kr, n0 : n0 + nn],
                        start=(k == 0),
                        stop=(k == KT - 1),
                    )
            return ps

        with tc.tile_pool(name="abp", bufs=cfg.ab_bufs) as abp, tc.tile_pool(
            name="pst", bufs=1, space="PSUM"
        ) as pst, tc.tile_pool(name="tstage", bufs=1) as tstage, tc.tile_pool(
            name="psw", bufs=2, space="PSUM"
        ) as psw, tc.tile_pool(name="wstage", bufs=2) as wstage:
            # ---- layer 1: H^T = relu(A @ (x W1))^T ----
            ps1 = spmm_T(ytab, HID, pst, abp, "l1")
            HT_s = tstage.tile([HID, R], bf16)
            nc.scalar.activation(
                HT_s[:, :], ps1[:, :], mybir.ActivationFunctionType.Relu
            )
            # Hw rows = (H @ W2)[m-tile] via lhsT = H^T slices (layout flip)
            for m in range(MT):
                rm = rows_of(m)
                pw = psw.tile([128, CODE], f32, space="PSUM")
                nc.tensor.matmul(
                    pw[:rm, :],
                    lhsT=HT_s[:, m * 128 : m * 128 + rm],
                    rhs=w2b[:, :],
                    start=True,
                    stop=True,
                )
                sw = wstage.tile([128, CODE], bf16)
                nc.vector.tensor_copy(sw[:rm, :], pw[:rm, :])
                nc.sync.dma_start(hw_own[m * 128 : m * 128 + rm, :], sw[:rm, :])

            nc.gpsimd.collective_compute(
                "AllGather",
                mybir.AluOpType.bypass,
                replica_groups=groups_all,
                ins=[hw_own.opt()],
                outs=[hw_all.opt()],
            )
            nc.sync.dma_start(
                htab[:, : KT - 1, :],
                hw_all[:nfull, :].rearrange("(k p) f -> p k f", p=128),
            )
            nc.sync.dma_start(htab[:KLAST, KT - 1, :], hw_all[nfull:, :])

            # ---- layer 2: z^T = (A @ Hw)^T  [CODE, R] ----
            ps2 = spmm_T(htab, CODE, pst, abp, "l2")
            zT_s = tstage.tile([CODE, R], bf16, tag="zts")
            nc.vector.tensor_copy(zT_s[:, :], ps2[:, :])
            nc.sync.dma_start(zt_own[:, :], zT_s[:, :])

        nc.gpsimd.collective_compute(
            "AllGather",
            mybir.AluOpType.bypass,
            replica_groups=groups_all,
            ins=[zt_own.opt()],
            outs=[zt_all.opt()],
        )
        # load z^T gathered into 4 partition strips
        for s in range(4):
            nc.sync.dma_start(
                ztall4[32 * s : 32 * s + CODE, :].rearrange(
                    "p (r j) -> p r j", r=cfg.n_cores
                ),
                zt_all.rearrange("r p j -> p r j"),
            )
            nc.sync.dma_start(zts4[32 * s : 32 * s + CODE, :], zt_own[:, :])

        # ================= decode =================
        with tc.tile_pool(name="obuf", bufs=2) as obuf, tc.tile_pool(
            name="psd", bufs=2, space="PSUM"
        ) as psd:
            qq = 0
            for m in range(MT):
                rm = rows_of(m)
                ob = obuf.tile([128, N], bf16)
                for bg in bank_groups:
                    w = sum(nn for _, nn in bg)
                    pd = psd.tile([128, 2048], f32, space="PSUM")
                    for q, (nn0, nn) in enumerate(bg):
                        s = qq % 4  # rotate PE row strips so LDW pipelines
                        qq += 1
                        p0 = 32 * s
                        nc.tensor.matmul(
                            pd[:rm, q * 512 : q * 512 + nn],
                            lhsT=zts4[p0 : p0 + CODE, m * 128 : m * 128 + rm],
                            rhs=ztall4[p0 : p0 + CODE, nn0 : nn0 + nn],
                            start=True,
                            stop=True,
                            tile_position=(p0, 0),
                        )
                    b0 = bg[0][0]
                    nc.scalar.activation(
                        ob[:rm, b0 : b0 + w],
                        pd[:rm, :w],
                        mybir.ActivationFunctionType.Sigmoid,
                    )
                nc.sync.dma_start(out_d[m * 128 : m * 128 + rm, :], ob[:rm, :])

    nc.compile()
    return nc


def _host_prep(cfg: Cfg, x, W1, W2, edge_weight, src, dst):
    x = np.asarray(x, dtype=np.float32)
    W1 = np.ascontiguousarray(np.asarray(W1, dtype=np.float32))
    W2 = np.ascontiguousarray(np.asarray(W2, dtype=np.float32))
    src = np.asarray(src).astype(np.int64)
    dst = np.asarray(dst).astype(np.int64)
    ew = np.asarray(edge_weight).astype(np.float64)
    R = cfg.rows
    in_maps = []
    for c in range(cfg.n_cores):
        lo = c * R
        m = (dst >= lo) & (dst < lo + R)
        # AB[s, j] = sum of edge weights s -> lo+j
        flat = src[m] * R + (dst[m] - lo)
        D = np.bincount(flat, weights=ew[m], minlength=cfg.n_nodes * R)
        ab = D.reshape(cfg.n_nodes, R).astype(ml_dtypes.bfloat16)
        in_maps.append(
            {
                "xt": np.ascontiguousarray(x[lo : lo + R].T),
                "w1": W1,
                "w2": W2,
                "ab": np.ascontiguousarray(ab),
            }
        )
    return in_maps


def kernel(x, W1, W2, edge_weight, src, dst, trace=False):
    cfg = Cfg()
    in_maps = _host_prep(cfg, x, W1, W2, edge_weight, src, dst)
    nc = build_nc(cfg)
    res = run_bass_kernel_spmd(
        nc, in_maps, core_ids=list(range(cfg.n_cores)), trace=trace
    )
    out = np.concatenate([r["out"] for r in res.results], axis=0)
    if trace:
        kernel.last_results = res
    return np.ascontiguousarray(out.astype(np.float32))


# revision 6
# speedup vs baseline: 1.9342x; 1.0164x over previous
"""GCN autoencoder kernel for 8 Trainium2 NeuronCores — dense-block SpMM.

Strategy (self-contained; shapes hardcoded for the graded problem):
  - Nodes row-sharded 1250/core. Host precomputes, per core, the dense
    adjacency slab AB[s, j] = A_hat[base+j, s] as 40 pair-packed bf16
    blocks [128, 2, 1250] (zero-padded to 80 k-tiles) — graph-constant
    layout prep. Host also ships x pre-transposed bf16 per core.
  - Y1 = x @ W1 computed row-wise per m-tile (lhsT = xT slabs).
    AllGather of unpadded bf16 Y1 rows [10000, 32].
  - SpMM layers run transposed on PE: out^T[F, 1250] = sum_k T_k^T @ AB_k
    with the tiny table k-tile as stationary weights and the dense AB
    k-block streamed from HBM as the moving operand (no per-edge DMA
    gather, no DVE one-hot builds, no GpSimd descriptor generation).
    The first 20 pairs stay resident in SBUF and are reused by layer 2;
    only the tail re-streams.
  - relu on ScalarE; Hw = H @ W2 fused with the layout flip back to rows
    (lhsT = H^T m-slices); AllGather Hw rows; layer 2 gives z^T [16, 1250]
    directly; AllGather z^T.
  - Decode: out = sigmoid(z_own @ z_all^T) bf16 matmuls (N=512 chunks,
    4-strip row rotation), ScalarE sigmoid PSUM->SBUF, bf16 output
    stored per 2048-col bank group (host casts back to f32).
"""

from contextlib import ExitStack
from dataclasses import dataclass

import numpy as np
import ml_dtypes

import concourse.bass as bass
import concourse.mybir as mybir
import concourse.tile as tile
from concourse import bacc
from concourse.bass_utils import run_bass_kernel_spmd

dt = mybir.dt


@dataclass
class Cfg:
    n_nodes: int = 10000
    n_feat: int = 512
    hid: int = 32
    code: int = 16
    n_cores: int = 8
    res_pairs: int = 20   # AB pairs kept resident in SBUF for layer-2 reuse
    str_bufs: int = 6     # streamed AB pairs in flight

    @property
    def rows(self):
        return self.n_nodes // self.n_cores

    @property
    def kt(self):  # padded 128-row k-tiles over the node axis
        return 80

    @property
    def qt(self):  # AB pairs
        return 40

    @property
    def mt(self):  # 128-row m-tiles per core
        return -(-self.rows // 128)

    @property
    def kc(self):  # 128-row chunks of n_feat
        return self.n_feat // 128


def _nchunks(total, step=512):
    out = []
    n0 = 0
    while n0 < total:
        out.append((n0, min(step, total - n0)))
        n0 += step
    return out


def build_nc(cfg: Cfg):
    nc = bacc.Bacc(
        "TRN2",
        target_bir_lowering=False,
        debug=False,
        enable_asserts=False,
        num_devices=cfg.n_cores,
    )
    f32 = dt.float32
    bf16 = dt.bfloat16
    N, R, HID, CODE = cfg.n_nodes, cfg.rows, cfg.hid, cfg.code
    KT, QT, MT, KC = cfg.kt, cfg.qt, cfg.mt, cfg.kc
    KFULL = N // 128          # 78 full k-tiles
    KLAST = N - KFULL * 128   # 16 real rows in tile 78

    # ---- external I/O ----
    xT_d = nc.dram_tensor("xt", [cfg.n_feat, R], bf16, kind="ExternalInput").ap()
    w1_d = nc.dram_tensor("w1", [cfg.n_feat, HID], f32, kind="ExternalInput").ap()
    w2_d = nc.dram_tensor("w2", [HID, CODE], f32, kind="ExternalInput").ap()
    ab_d = nc.dram_tensor("ab", [QT, 128, 2 * R], bf16, kind="ExternalInput").ap()
    out_d = nc.dram_tensor("out", [R, N], bf16, kind="ExternalOutput").ap()

    # ---- internal DRAM ----
    y1_own = nc.dram_tensor("y1_own", [R, HID], bf16).ap()
    y1_all = nc.dram_tensor("y1_all", [N, HID], bf16, addr_space="Shared").ap()
    hw_own = nc.dram_tensor("hw_own", [R, CODE], bf16).ap()
    hw_all = nc.dram_tensor("hw_all", [N, CODE], bf16, addr_space="Shared").ap()
    zt_own = nc.dram_tensor("zt_own", [CODE, R], bf16).ap()
    zt_all = nc.dram_tensor(
        "zt_all", [cfg.n_cores, CODE, R], bf16, addr_space="Shared"
    ).ap()

    groups_all = [list(range(cfg.n_cores))]
    rchunks = _nchunks(R)          # psum n-chunking over the 1250 dst cols

    def rows_of(m):
        return min(128, R - m * 128)

    # decode N-chunking: 512-wide chunks grouped 4 per PSUM tile
    bank_groups = []
    ncs = _nchunks(N)
    for i in range(0, len(ncs), 4):
        bank_groups.append(ncs[i : i + 4])

    with tile.TileContext(nc) as tc, ExitStack() as ctx:
        cpool = ctx.enter_context(tc.tile_pool(name="consts", bufs=1))
        zpool = ctx.enter_context(tc.tile_pool(name="zbits", bufs=1))
        resp = ctx.enter_context(tc.tile_pool(name="abres", bufs=cfg.res_pairs))
        strp = ctx.enter_context(tc.tile_pool(name="abstr", bufs=cfg.str_bufs))
        tabp = ctx.enter_context(tc.tile_pool(name="tab", bufs=1))

        w1s = cpool.tile([128, KC, HID], bf16)
        w1f = cpool.tile([128, KC, HID], f32)
        for k in range(KC):
            nc.sync.dma_start(w1f[:, k, :], w1_d[k * 128 : (k + 1) * 128, :])
        nc.vector.tensor_copy(w1s[:], w1f[:])
        w2s = cpool.tile([HID, CODE], f32)
        nc.sync.dma_start(w2s[:], w2_d[:, :])
        w2b = cpool.tile([HID, CODE], bf16)
        nc.vector.tensor_copy(w2b[:], w2s[:])

        # decode operands replicated at 4 partition strips
        zts4 = zpool.tile([128, R], bf16)
        ztall4 = zpool.tile([128, N], bf16)

        def load_table(fdim, src_all, tag):
            """[10000, F] row table -> [128, KT, F] k-tiled, zero-padded."""
            tab = tabp.tile([128, KT, max(HID, CODE)], bf16, tag="tab")
            t = tab[:, :, :fdim]
            nc.vector.memset(t[:, KFULL : KFULL + 2, :], 0.0)
            nc.sync.dma_start(
                t[:, :KFULL, :],
                src_all[: KFULL * 128, :].rearrange("(k p) f -> p k f", p=128),
            )
            nc.sync.dma_start(t[:KLAST, KFULL, :], src_all[KFULL * 128 :, :])
            return t

        # ================= phase Y1: y1 = x @ W1 (row layout) ============
        with tc.tile_pool(name="xts", bufs=1) as xtp, tc.tile_pool(
            name="psy", bufs=4, space="PSUM"
        ) as psy, tc.tile_pool(name="ystage", bufs=4) as ystage:
            xTs = xtp.tile([128, KC, R], bf16)
            nc.sync.dma_start(
                xTs[:, :, :], xT_d.rearrange("(k p) n -> p k n", p=128)
            )
            for m in range(MT):
                rm = rows_of(m)
                py = psy.tile([128, HID], f32, space="PSUM")
                for k in range(KC):
                    nc.tensor.matmul(
                        py[:rm, :],
                        lhsT=xTs[:, k, m * 128 : m * 128 + rm],
                        rhs=w1s[:, k, :],
                        start=(k == 0),
                        stop=(k == KC - 1),
                    )
                st = ystage.tile([128, HID], bf16)
                nc.vector.tensor_copy(st[:rm, :], py[:rm, :])
                nc.sync.dma_start(y1_own[m * 128 : m * 128 + rm, :], st[:rm, :])

        nc.gpsimd.collective_compute(
            "AllGather",
            mybir.AluOpType.bypass,
            replica_groups=groups_all,
            ins=[y1_own.opt()],
            outs=[y1_all.opt()],
        )

        # ================= dense SpMM layers =================
        ab_tiles = {}

        def ab_tile(q, layer1):
            if q < cfg.res_pairs:
                if layer1:
                    t = resp.tile([128, 2, R], bf16, tag="abres")
                    nc.sync.dma_start(t[:, :, :], ab_d[q].rearrange("p (l n) -> p l n", l=2))
                    ab_tiles[q] = t
                return ab_tiles[q]
            t = strp.tile([128, 2, R], bf16, tag="abstr")
            nc.sync.dma_start(t[:, :, :], ab_d[q].rearrange("p (l n) -> p l n", l=2))
            return t

        def spmm_T(tab, fdim, pst, layer1, tag):
            """psum[fdim, R] = sum_k tab_k^T @ AB_k  (pair-packed blocks)."""
            ps = pst.tile([fdim, R], f32, space="PSUM", tag=f"ps_{tag}")
            for q in range(QT):
                ab = ab_tile(q, layer1)
                for l in range(2):
                    k = 2 * q + l
                    for n0, nn in rchunks:
                        nc.tensor.matmul(
                            ps[:, n0 : n0 + nn],
                            lhsT=tab[:, k, :],
                            rhs=ab[:, l, n0 : n0 + nn],
                            start=(k == 0),
                            stop=(k == KT - 1),
                        )
            return ps

        with tc.tile_pool(name="pst", bufs=1, space="PSUM") as pst, tc.tile_pool(
            name="tstage", bufs=1
        ) as tstage, tc.tile_pool(name="psw", bufs=2, space="PSUM") as psw, tc.tile_pool(
            name="wstage", bufs=4
        ) as wstage:
            # ---- layer 1: H^T = relu(A @ (x W1))^T ----
            ytab = load_table(HID, y1_all, "y")
            ps1 = spmm_T(ytab, HID, pst, True, "l1")
            HT_s = tstage.tile([HID, R], bf16)
            nc.scalar.activation(
                HT_s[:, :], ps1[:, :], mybir.ActivationFunctionType.Relu
            )
            # Hw rows = (H @ W2)[m-tile] via lhsT = H^T slices (layout flip)
            for m in range(MT):
                rm = rows_of(m)
                pw = psw.tile([128, CODE], f32, space="PSUM")
                nc.tensor.matmul(
                    pw[:rm, :],
                    lhsT=HT_s[:, m * 128 : m * 128 + rm],
                    rhs=w2b[:, :],
                    start=True,
                    stop=True,
                )
                sw = wstage.tile([128, CODE], bf16)
                nc.vector.tensor_copy(sw[:rm, :], pw[:rm, :])
                nc.sync.dma_start(hw_own[m * 128 : m * 128 + rm, :], sw[:rm, :])

            nc.gpsimd.collective_compute(
                "AllGather",
                mybir.AluOpType.bypass,
                replica_groups=groups_all,
                ins=[hw_own.opt()],
                outs=[hw_all.opt()],
            )

            # ---- layer 2: z^T = (A @ Hw)^T  [CODE, R] ----
            htab = load_table(CODE, hw_all, "h")
            ps2 = spmm_T(htab, CODE, pst, False, "l2")
            zT_s = tstage.tile([CODE, R], bf16, tag="zts")
            nc.vector.tensor_copy(zT_s[:, :], ps2[:, :])
            nc.sync.dma_start(zt_own[:, :], zT_s[:, :])
            # own-z decode operand can stage before the AllGather
            for s in range(4):
                nc.vector.tensor_copy(zts4[32 * s : 32 * s + CODE, :], zT_s[:, :])

        nc.gpsimd.collective_compute(
            "AllGather",
            mybir.AluOpType.bypass,
            replica_groups=groups_all,
            ins=[zt_own.opt()],
            outs=[zt_all.opt()],
        )
        # load z^T gathered into 4 partition strips
        for s in range(4):
            nc.sync.dma_start(
                ztall4[32 * s : 32 * s + CODE, :].rearrange(
                    "p (r j) -> p r j", r=cfg.n_cores
                ),
                zt_all.rearrange("r p j -> p r j"),
            )

        # ================= decode =================
        with tc.tile_pool(name="obuf", bufs=4) as obuf, tc.tile_pool(
            name="psd", bufs=2, space="PSUM"
        ) as psd:
            qq = 0
            for m in range(MT):
                rm = rows_of(m)
                for bg in bank_groups:
                    w = sum(nn for _, nn in bg)
                    pd = psd.tile([128, 2048], f32, space="PSUM")
                    for q, (nn0, nn) in enumerate(bg):
                        s = qq % 4  # rotate PE row strips so LDW pipelines
                        qq += 1
                        p0 = 32 * s
                        nc.tensor.matmul(
                            pd[:rm, q * 512 : q * 512 + nn],
                            lhsT=zts4[p0 : p0 + CODE, m * 128 : m * 128 + rm],
                            rhs=ztall4[p0 : p0 + CODE, nn0 : nn0 + nn],
                            start=True,
                            stop=True,
                            tile_position=(p0, 0),
                        )
                    ob = obuf.tile([128, 2048], bf16)
                    nc.scalar.activation(
                        ob[:rm, :w],
                        pd[:rm, :w],
                        mybir.ActivationFunctionType.Sigmoid,
                    )
                    b0 = bg[0][0]
                    nc.sync.dma_start(
                        out_d[m * 128 : m * 128 + rm, b0 : b0 + w], ob[:rm, :w]
                    )

    nc.compile()
    return nc


def _host_prep(cfg: Cfg, x, W1, W2, edge_weight, src, dst):
    x = np.asarray(x, dtype=np.float32)
    W1 = np.ascontiguousarray(np.asarray(W1, dtype=np.float32))
    W2 = np.ascontiguousarray(np.asarray(W2, dtype=np.float32))
    src = np.asarray(src).astype(np.int64)
    dst = np.asarray(dst).astype(np.int64)
    ew = np.asarray(edge_weight).astype(np.float64)
    R = cfg.rows
    NPAD = cfg.kt * 128
    in_maps = []
    for c in range(cfg.n_cores):
        lo = c * R
        m = (dst >= lo) & (dst < lo + R)
        # AB[s, j] = sum of edge weights s -> lo+j, zero-padded node axis
        flat = src[m] * R + (dst[m] - lo)
        D = np.zeros(NPAD * R, np.float32)
        D[: cfg.n_nodes * R] = np.bincount(
            flat, weights=ew[m], minlength=cfg.n_nodes * R
        )
        # pair-pack: [80, 128, R] -> [40, 128, 2*R]
        ab = (
            D.reshape(cfg.qt, 2, 128, R)
            .transpose(0, 2, 1, 3)
            .reshape(cfg.qt, 128, 2 * R)
            .astype(ml_dtypes.bfloat16)
        )
        in_maps.append(
            {
                "xt": np.ascontiguousarray(
                    x[lo : lo + R].T.astype(ml_dtypes.bfloat16)
                ),
                "w1": W1,
                "w2": W2,
                "ab": np.ascontiguousarray(ab),
            }
        )
    return in_maps


def kernel(x, W1, W2, edge_weight, src, dst, trace=False):
    cfg = Cfg()
    in_maps = _host_prep(cfg, x, W1, W2, edge_weight, src, dst)
    nc = build_nc(cfg)
    res = run_bass_kernel_spmd(
        nc, in_maps, core_ids=list(range(cfg.n_cores)), trace=trace
    )
    out = np.concatenate([r["out"] for r in res.results], axis=0)
    if trace:
        kernel.last_results = res
    return np.ascontiguousarray(out.astype(np.float32))


# revision 7
# speedup vs baseline: 2.0963x; 1.0838x over previous
"""GCN autoencoder kernel for 8 Trainium2 NeuronCores — dense-block SpMM.

Strategy (self-contained; shapes hardcoded for the graded problem):
  - Nodes row-sharded 1250/core, padded to 1280/core (padded ids
    n' = 1280c + i). Contraction tiles use the permuted layout
    node(p, k) = 80p + k so the gathered feature tables load as one
    contiguous stripe per partition.
  - Host precomputes, per core, the dense adjacency slab
    AB[n', j] = A_hat[base+j, n'] as 40 pair-packed bf16 blocks
    [128, 2, 1250] in the permuted row order — graph-constant layout
    prep. Host also ships x pre-transposed bf16 (zero-padded).
  - Y1 = x @ W1 computed row-wise per m-tile (lhsT = xT slabs).
    AllGather of bf16 Y1 rows [10240, 32].
  - SpMM layers run transposed on PE: out^T[F, 1250] = sum_k T_k^T @ AB_k
    with the table k-tile as stationary weights and the dense AB block
    streamed from HBM as the moving operand (no per-edge DMA gather).
    AB streaming uses the Scalar-engine HWDGE ring so it cannot queue
    ahead of critical-path Sync-ring DMAs. The first 18 pairs stay
    resident in SBUF and are reused by layer 2; only the tail re-streams.
  - relu on ScalarE; Hw = H @ W2 fused with the layout flip back to rows
    (lhsT = H^T m-slices); AllGather Hw rows; layer 2 gives z^T [16, 1250]
    directly; AllGather z^T.
  - Decode: out = sigmoid(z_own @ z_all^T) bf16 matmuls (N=512 chunks,
    4-strip row rotation), ScalarE sigmoid PSUM->SBUF, bf16 output
    stored per 2048-col bank group (host casts back to f32).
"""

from contextlib import ExitStack
from dataclasses import dataclass

import numpy as np
import ml_dtypes

import concourse.bass as bass
import concourse.mybir as mybir
import concourse.tile as tile
from concourse import bacc
from concourse.bass_utils import run_bass_kernel_spmd

dt = mybir.dt


@dataclass
class Cfg:
    n_nodes: int = 10000
    n_feat: int = 512
    hid: int = 32
    code: int = 16
    n_cores: int = 8
    res_pairs: int = 18   # AB pairs kept resident in SBUF for layer-2 reuse
    str_bufs: int = 6     # streamed AB pairs in flight

    @property
    def rows(self):
        return self.n_nodes // self.n_cores          # 1250 real rows/core

    @property
    def rpad(self):
        return 1280                                   # padded rows/core

    @property
    def npad(self):
        return self.rpad * self.n_cores               # 10240

    @property
    def kt(self):
        return 80                                     # k-tiles (npad/128)

    @property
    def qt(self):
        return 40                                     # AB pairs

    @property
    def mt(self):
        return self.rpad // 128                       # 10 m-tiles/core

    @property
    def kc(self):
        return self.n_feat // 128


def _nchunks(total, step=512):
    out = []
    n0 = 0
    while n0 < total:
        out.append((n0, min(step, total - n0)))
        n0 += step
    return out


def build_nc(cfg: Cfg):
    nc = bacc.Bacc(
        "TRN2",
        target_bir_lowering=False,
        debug=False,
        enable_asserts=False,
        num_devices=cfg.n_cores,
    )
    f32 = dt.float32
    bf16 = dt.bfloat16
    N, R, RP, NP = cfg.n_nodes, cfg.rows, cfg.rpad, cfg.npad
    HID, CODE = cfg.hid, cfg.code
    KT, QT, MT, KC = cfg.kt, cfg.qt, cfg.mt, cfg.kc

    # ---- external I/O ----
    xT_d = nc.dram_tensor("xt", [cfg.n_feat, RP], bf16, kind="ExternalInput").ap()
    w1_d = nc.dram_tensor("w1", [cfg.n_feat, HID], f32, kind="ExternalInput").ap()
    w2_d = nc.dram_tensor("w2", [HID, CODE], f32, kind="ExternalInput").ap()
    ab_d = nc.dram_tensor("ab", [QT, 128, 2 * R], bf16, kind="ExternalInput").ap()
    out_d = nc.dram_tensor("out", [R, N], bf16, kind="ExternalOutput").ap()

    # ---- internal DRAM ----
    y1_own = nc.dram_tensor("y1_own", [RP, HID], bf16).ap()
    y1_all = nc.dram_tensor("y1_all", [NP, HID], bf16, addr_space="Shared").ap()
    hw_own = nc.dram_tensor("hw_own", [RP, CODE], bf16).ap()
    hw_all = nc.dram_tensor("hw_all", [NP, CODE], bf16, addr_space="Shared").ap()
    zt_own = nc.dram_tensor("zt_own", [CODE, R], bf16).ap()
    zt_all = nc.dram_tensor(
        "zt_all", [cfg.n_cores, CODE, R], bf16, addr_space="Shared"
    ).ap()

    groups_all = [list(range(cfg.n_cores))]
    rchunks = _nchunks(R)          # psum n-chunking over the 1250 dst cols

    # decode N-chunking: 512-wide chunks grouped 4 per PSUM tile
    bank_groups = []
    ncs = _nchunks(N)
    for i in range(0, len(ncs), 4):
        bank_groups.append(ncs[i : i + 4])

    with tile.TileContext(nc) as tc, ExitStack() as ctx:
        cpool = ctx.enter_context(tc.tile_pool(name="consts", bufs=1))
        zpool = ctx.enter_context(tc.tile_pool(name="zbits", bufs=1))
        resp = ctx.enter_context(tc.tile_pool(name="abres", bufs=cfg.res_pairs))
        strp = ctx.enter_context(tc.tile_pool(name="abstr", bufs=cfg.str_bufs))
        tabp = ctx.enter_context(tc.tile_pool(name="tab", bufs=1))

        w1s = cpool.tile([128, KC, HID], bf16)
        w1f = cpool.tile([128, KC, HID], f32)
        for k in range(KC):
            nc.sync.dma_start(w1f[:, k, :], w1_d[k * 128 : (k + 1) * 128, :])
        nc.vector.tensor_copy(w1s[:], w1f[:])
        w2s = cpool.tile([HID, CODE], f32)
        nc.sync.dma_start(w2s[:], w2_d[:, :])
        w2b = cpool.tile([HID, CODE], bf16)
        nc.vector.tensor_copy(w2b[:], w2s[:])

        # decode operands replicated at 4 partition strips
        zts4 = zpool.tile([128, R], bf16)
        ztall4 = zpool.tile([128, N], bf16)

        def load_table(fdim, src_all):
            """[10240, F] row table -> [128, KT, F]: node 80p+k at (p, k)."""
            tab = tabp.tile([128, KT, max(HID, CODE)], bf16, tag="tab")
            t = tab[:, :, :fdim]
            nc.sync.dma_start(
                t[:, :, :], src_all.rearrange("(p k) f -> p k f", p=128)
            )
            return t

        # ================= phase Y1: y1 = x @ W1 (row layout) ============
        with tc.tile_pool(name="xts", bufs=1) as xtp, tc.tile_pool(
            name="psy", bufs=4, space="PSUM"
        ) as psy, tc.tile_pool(name="ystage", bufs=4) as ystage:
            xTs = xtp.tile([128, KC, RP], bf16)
            nc.sync.dma_start(
                xTs[:, :, :], xT_d.rearrange("(k p) n -> p k n", p=128)
            )
            for m in range(MT):
                py = psy.tile([128, HID], f32, space="PSUM")
                for k in range(KC):
                    nc.tensor.matmul(
                        py[:, :],
                        lhsT=xTs[:, k, m * 128 : (m + 1) * 128],
                        rhs=w1s[:, k, :],
                        start=(k == 0),
                        stop=(k == KC - 1),
                    )
                st = ystage.tile([128, HID], bf16)
                nc.vector.tensor_copy(st[:, :], py[:, :])
                nc.sync.dma_start(y1_own[m * 128 : (m + 1) * 128, :], st[:, :])

        nc.gpsimd.collective_compute(
            "AllGather",
            mybir.AluOpType.bypass,
            replica_groups=groups_all,
            ins=[y1_own.opt()],
            outs=[y1_all.opt()],
        )

        # ================= dense SpMM layers =================
        ab_tiles = {}

        def ab_tile(q, layer1):
            if q < cfg.res_pairs:
                if layer1:
                    t = resp.tile([128, 2, R], bf16, tag="abres")
                    nc.scalar.dma_start(
                        t[:, :, :], ab_d[q].rearrange("p (l n) -> p l n", l=2)
                    )
                    ab_tiles[q] = t
                return ab_tiles[q]
            t = strp.tile([128, 2, R], bf16, tag="abstr")
            nc.scalar.dma_start(
                t[:, :, :], ab_d[q].rearrange("p (l n) -> p l n", l=2)
            )
            return t

        def spmm_T(tab, fdim, pst, layer1, tag):
            """psum[fdim, R] = sum_k tab_k^T @ AB_k  (pair-packed blocks)."""
            ps = pst.tile([fdim, R], f32, space="PSUM", tag=f"ps_{tag}")
            for q in range(QT):
                ab = ab_tile(q, layer1)
                for l in range(2):
                    k = 2 * q + l
                    for n0, nn in rchunks:
                        nc.tensor.matmul(
                            ps[:, n0 : n0 + nn],
                            lhsT=tab[:, k, :],
                            rhs=ab[:, l, n0 : n0 + nn],
                            start=(k == 0),
                            stop=(k == KT - 1),
                        )
            return ps

        with tc.tile_pool(name="pst", bufs=1, space="PSUM") as pst, tc.tile_pool(
            name="tstage", bufs=1
        ) as tstage, tc.tile_pool(name="psw", bufs=2, space="PSUM") as psw, tc.tile_pool(
            name="wstage", bufs=4
        ) as wstage:
            # ---- layer 1: H^T = relu(A @ (x W1))^T ----
            ytab = load_table(HID, y1_all)
            ps1 = spmm_T(ytab, HID, pst, True, "l1")
            HT_s = tstage.tile([HID, RP], bf16)
            nc.vector.memset(HT_s[:, R:RP], 0.0)
            nc.scalar.activation(
                HT_s[:, :R], ps1[:, :], mybir.ActivationFunctionType.Relu
            )
            # Hw rows = (H @ W2)[m-tile] via lhsT = H^T slices (layout flip)
            for m in range(MT):
                pw = psw.tile([128, CODE], f32, space="PSUM")
                nc.tensor.matmul(
                    pw[:, :],
                    lhsT=HT_s[:, m * 128 : (m + 1) * 128],
                    rhs=w2b[:, :],
                    start=True,
                    stop=True,
                )
                sw = wstage.tile([128, CODE], bf16)
                nc.vector.tensor_copy(sw[:, :], pw[:, :])
                nc.sync.dma_start(hw_own[m * 128 : (m + 1) * 128, :], sw[:, :])

            nc.gpsimd.collective_compute(
                "AllGather",
                mybir.AluOpType.bypass,
                replica_groups=groups_all,
                ins=[hw_own.opt()],
                outs=[hw_all.opt()],
            )

            # ---- layer 2: z^T = (A @ Hw)^T  [CODE, R] ----
            htab = load_table(CODE, hw_all)
            ps2 = spmm_T(htab, CODE, pst, False, "l2")
            zT_s = tstage.tile([CODE, R], bf16, tag="zts")
            nc.vector.tensor_copy(zT_s[:, :], ps2[:, :])
            nc.sync.dma_start(zt_own[:, :], zT_s[:, :])
            # own-z decode operand can stage before the AllGather
            for s in range(4):
                nc.vector.tensor_copy(zts4[32 * s : 32 * s + CODE, :], zT_s[:, :])

        nc.gpsimd.collective_compute(
            "AllGather",
            mybir.AluOpType.bypass,
            replica_groups=groups_all,
            ins=[zt_own.opt()],
            outs=[zt_all.opt()],
        )
        # load z^T gathered into 4 partition strips
        for s in range(4):
            nc.sync.dma_start(
                ztall4[32 * s : 32 * s + CODE, :].rearrange(
                    "p (r j) -> p r j", r=cfg.n_cores
                ),
                zt_all.rearrange("r p j -> p r j"),
            )

        # ================= decode =================
        with tc.tile_pool(name="obuf", bufs=4) as obuf, tc.tile_pool(
            name="psd", bufs=2, space="PSUM"
        ) as psd:
            qq = 0
            for m in range(MT):
                rm = min(128, R - m * 128)
                if rm <= 0:
                    continue
                for bg in bank_groups:
                    w = sum(nn for _, nn in bg)
                    pd = psd.tile([128, 2048], f32, space="PSUM")
                    for q, (nn0, nn) in enumerate(bg):
                        s = qq % 4  # rotate PE row strips so LDW pipelines
                        qq += 1
                        p0 = 32 * s
                        nc.tensor.matmul(
                            pd[:rm, q * 512 : q * 512 + nn],
                            lhsT=zts4[p0 : p0 + CODE, m * 128 : m * 128 + rm],
                            rhs=ztall4[p0 : p0 + CODE, nn0 : nn0 + nn],
                            start=True,
                            stop=True,
                            tile_position=(p0, 0),
                        )
                    ob = obuf.tile([128, 2048], bf16)
                    nc.scalar.activation(
                        ob[:rm, :w],
                        pd[:rm, :w],
                        mybir.ActivationFunctionType.Sigmoid,
                    )
                    b0 = bg[0][0]
                    nc.sync.dma_start(
                        out_d[m * 128 : m * 128 + rm, b0 : b0 + w], ob[:rm, :w]
                    )

    nc.compile()
    return nc


def _host_prep(cfg: Cfg, x, W1, W2, edge_weight, src, dst):
    x = np.asarray(x, dtype=np.float32)
    W1 = np.ascontiguousarray(np.asarray(W1, dtype=np.float32))
    W2 = np.ascontiguousarray(np.asarray(W2, dtype=np.float32))
    src = np.asarray(src).astype(np.int64)
    dst = np.asarray(dst).astype(np.int64)
    ew = np.asarray(edge_weight).astype(np.float64)
    R, RP = cfg.rows, cfg.rpad
    # padded node id: n' = 1280*(s//1250) + s%1250
    srcp = RP * (src // R) + (src % R)
    in_maps = []
    for c in range(cfg.n_cores):
        lo = c * R
        m = (dst >= lo) & (dst < lo + R)
        # AB[n', j] = sum of edge weights src -> lo+j, permuted node axis
        flat = srcp[m] * R + (dst[m] - lo)
        D = np.bincount(flat, weights=ew[m], minlength=cfg.npad * R).astype(
            np.float32
        )
        # permuted pair-pack: block k rows are nodes 80p+k ->
        # D[(p k) j] -> [q=k/2, p, l=k%2, j]
        ab = (
            D.reshape(128, cfg.qt, 2, R)
            .transpose(1, 0, 2, 3)
            .reshape(cfg.qt, 128, 2 * R)
            .astype(ml_dtypes.bfloat16)
        )
        xpad = np.zeros((RP, cfg.n_feat), np.float32)
        xpad[:R] = x[lo : lo + R]
        in_maps.append(
            {
                "xt": np.ascontiguousarray(xpad.T.astype(ml_dtypes.bfloat16)),
                "w1": W1,
                "w2": W2,
                "ab": np.ascontiguousarray(ab),
            }
        )
    return in_maps


def kernel(x, W1, W2, edge_weight, src, dst, trace=False):
    cfg = Cfg()
    in_maps = _host_prep(cfg, x, W1, W2, edge_weight, src, dst)
    nc = build_nc(cfg)
    res = run_bass_kernel_spmd(
        nc, in_maps, core_ids=list(range(cfg.n_cores)), trace=trace
    )
    out = np.concatenate([r["out"] for r in res.results], axis=0)
    if trace:
        kernel.last_results = res
    return np.ascontiguousarray(out.astype(np.float32))


# revision 8
# speedup vs baseline: 2.1865x; 1.0430x over previous
"""GCN autoencoder kernel for 8 Trainium2 NeuronCores — dense-block SpMM.

Strategy (self-contained; shapes hardcoded for the graded problem):
  - Nodes row-sharded 1250/core, padded to 1280/core (padded ids
    n' = 1280c + i). Contraction tiles use the permuted layout
    node(p, k) = 80p + k so the gathered feature tables load as one
    contiguous stripe per partition.
  - Host precomputes, per core, the dense adjacency slab
    AB[n', j] = A_hat[base+j, n'] as 40 pair-packed bf16 blocks
    [128, 2, 1250] in the permuted row order — graph-constant layout
    prep. Host also ships x pre-transposed bf16 (zero-padded).
  - Y1 = x @ W1 computed row-wise per m-tile (lhsT = xT slabs).
    AllGather of bf16 Y1 rows [10240, 32].
  - SpMM layers run transposed on PE: out^T[F, 1250] = sum_k T_k^T @ AB_k
    with the table k-tile as stationary weights and the dense AB block
    streamed from HBM as the moving operand (no per-edge DMA gather).
    AB streaming uses the Scalar-engine HWDGE ring so it cannot queue
    ahead of critical-path Sync-ring DMAs. The first 18 pairs stay
    resident in SBUF and are reused by layer 2; only the tail re-streams.
  - relu on ScalarE; Hw = H @ W2 fused with the layout flip back to rows
    (lhsT = H^T m-slices); AllGather Hw rows; layer 2 gives z^T [16, 1250]
    directly; AllGather z^T.
  - Decode: out = sigmoid(z_own @ z_all^T) bf16 matmuls (N=512 chunks,
    4-strip row rotation), ScalarE sigmoid PSUM->SBUF, bf16 output
    stored per 2048-col bank group (host casts back to f32).
"""

from contextlib import ExitStack
from dataclasses import dataclass

import numpy as np
import ml_dtypes

import concourse.bass as bass
import concourse.mybir as mybir
import concourse.tile as tile
from concourse import bacc
from concourse.bass_utils import run_bass_kernel_spmd

dt = mybir.dt


@dataclass
class Cfg:
    n_nodes: int = 10000
    n_feat: int = 512
    hid: int = 32
    code: int = 16
    n_cores: int = 8
    res_pairs: int = 40   # all AB pairs resident in SBUF (fp8), reused by layer 2

    @property
    def rows(self):
        return self.n_nodes // self.n_cores          # 1250 real rows/core

    @property
    def rpad(self):
        return 1280                                   # padded rows/core

    @property
    def npad(self):
        return self.rpad * self.n_cores               # 10240

    @property
    def kt(self):
        return 80                                     # k-tiles (npad/128)

    @property
    def qt(self):
        return 40                                     # AB pairs

    @property
    def mt(self):
        return self.rpad // 128                       # 10 m-tiles/core

    @property
    def kc(self):
        return self.n_feat // 128


def _nchunks(total, step=512):
    out = []
    n0 = 0
    while n0 < total:
        out.append((n0, min(step, total - n0)))
        n0 += step
    return out


def build_nc(cfg: Cfg):
    nc = bacc.Bacc(
        "TRN2",
        target_bir_lowering=False,
        debug=False,
        enable_asserts=False,
        num_devices=cfg.n_cores,
    )
    f32 = dt.float32
    bf16 = dt.bfloat16
    N, R, RP, NP = cfg.n_nodes, cfg.rows, cfg.rpad, cfg.npad
    HID, CODE = cfg.hid, cfg.code
    KT, QT, MT, KC = cfg.kt, cfg.qt, cfg.mt, cfg.kc

    # ---- external I/O ----
    xT_d = nc.dram_tensor("xt", [cfg.n_feat, RP], bf16, kind="ExternalInput").ap()
    w1_d = nc.dram_tensor("w1", [cfg.n_feat, HID], bf16, kind="ExternalInput").ap()
    w2_d = nc.dram_tensor("w2", [HID, CODE], bf16, kind="ExternalInput").ap()
    ab_d = nc.dram_tensor("ab", [QT, 128, 2 * R], dt.float8e4, kind="ExternalInput").ap()
    out_d = nc.dram_tensor("out", [R, N], bf16, kind="ExternalOutput").ap()

    # ---- internal DRAM ----
    y1_own = nc.dram_tensor("y1_own", [RP, HID], bf16).ap()
    y1_all = nc.dram_tensor("y1_all", [NP, HID], bf16, addr_space="Shared").ap()
    hw_own = nc.dram_tensor("hw_own", [RP, CODE], bf16).ap()
    hw_all = nc.dram_tensor("hw_all", [NP, CODE], bf16, addr_space="Shared").ap()
    zt_own = nc.dram_tensor("zt_own", [CODE, R], bf16).ap()
    zt_all = nc.dram_tensor(
        "zt_all", [cfg.n_cores, CODE, R], bf16, addr_space="Shared"
    ).ap()

    groups_all = [list(range(cfg.n_cores))]
    rchunks = _nchunks(R)          # psum n-chunking over the 1250 dst cols

    # decode N-chunking: 512-wide chunks grouped 4 per PSUM tile
    bank_groups = []
    ncs = _nchunks(N)
    for i in range(0, len(ncs), 4):
        bank_groups.append(ncs[i : i + 4])

    with tile.TileContext(nc) as tc, ExitStack() as ctx:
        cpool = ctx.enter_context(tc.tile_pool(name="consts", bufs=1))
        zpool = ctx.enter_context(tc.tile_pool(name="zbits", bufs=1))
        resp = ctx.enter_context(tc.tile_pool(name="abres", bufs=cfg.res_pairs))
        tabp = ctx.enter_context(tc.tile_pool(name="tab", bufs=1))

        w1s = cpool.tile([128, KC, HID], bf16)
        for k in range(KC):
            nc.sync.dma_start(w1s[:, k, :], w1_d[k * 128 : (k + 1) * 128, :])
        w2b = cpool.tile([HID, CODE], bf16)
        nc.sync.dma_start(w2b[:], w2_d[:, :])

        # decode operands replicated at 4 partition strips
        zts4 = zpool.tile([128, R], bf16)
        ztall4 = zpool.tile([128, N], bf16)

        def load_table(fdim, src_all):
            """[10240, F] row table -> [128, KT, F]: node 80p+k at (p, k)."""
            tab = tabp.tile([128, KT, max(HID, CODE)], bf16, tag="tab")
            t = tab[:, :, :fdim]
            nc.sync.dma_start(
                t[:, :, :], src_all.rearrange("(p k) f -> p k f", p=128)
            )
            return t

        # ================= phase Y1: y1 = x @ W1 (row layout) ============
        with tc.tile_pool(name="xts", bufs=1) as xtp, tc.tile_pool(
            name="psy", bufs=4, space="PSUM"
        ) as psy, tc.tile_pool(name="ystage", bufs=4) as ystage:
            xTs = xtp.tile([128, KC, RP], bf16)
            nc.sync.dma_start(
                xTs[:, :, :], xT_d.rearrange("(k p) n -> p k n", p=128)
            )
            for m in range(MT):
                py = psy.tile([128, HID], f32, space="PSUM")
                for k in range(KC):
                    nc.tensor.matmul(
                        py[:, :],
                        lhsT=xTs[:, k, m * 128 : (m + 1) * 128],
                        rhs=w1s[:, k, :],
                        start=(k == 0),
                        stop=(k == KC - 1),
                    )
                st = ystage.tile([128, HID], bf16)
                nc.vector.tensor_copy(st[:, :], py[:, :])
                nc.sync.dma_start(y1_own[m * 128 : (m + 1) * 128, :], st[:, :])

        nc.gpsimd.collective_compute(
            "AllGather",
            mybir.AluOpType.bypass,
            replica_groups=groups_all,
            ins=[y1_own.opt()],
            outs=[y1_all.opt()],
        )

        # ================= dense SpMM layers =================
        ab_tiles = {}

        def ab_tile(q, layer1):
            if layer1:
                t = resp.tile([128, 2, R], dt.float8e4, tag="abres")
                nc.scalar.dma_start(
                    t[:, :, :], ab_d[q].rearrange("p (l n) -> p l n", l=2)
                )
                ab_tiles[q] = t
            return ab_tiles[q]

        def spmm_T(tab, fdim, pst, layer1, tag):
            """psum[fdim, R] = sum_k tab_k^T @ AB_k  (pair-packed blocks)."""
            ps = pst.tile([fdim, R], f32, space="PSUM", tag=f"ps_{tag}")
            for q in range(QT):
                ab = ab_tile(q, layer1)
                for l in range(2):
                    k = 2 * q + l
                    for n0, nn in rchunks:
                        nc.tensor.matmul(
                            ps[:, n0 : n0 + nn],
                            lhsT=tab[:, k, :],
                            rhs=ab[:, l, n0 : n0 + nn],
                            start=(k == 0),
                            stop=(k == KT - 1),
                        )
            return ps

        with tc.tile_pool(name="pst", bufs=1, space="PSUM") as pst, tc.tile_pool(
            name="tstage", bufs=1
        ) as tstage, tc.tile_pool(name="psw", bufs=2, space="PSUM") as psw, tc.tile_pool(
            name="wstage", bufs=4
        ) as wstage:
            # ---- layer 1: H^T = relu(A @ (x W1))^T ----
            ytab = load_table(HID, y1_all)
            ps1 = spmm_T(ytab, HID, pst, True, "l1")
            HT_s = tstage.tile([HID, RP], bf16)
            nc.vector.memset(HT_s[:, R:RP], 0.0)
            nc.scalar.activation(
                HT_s[:, :R], ps1[:, :], mybir.ActivationFunctionType.Relu
            )
            # Hw rows = (H @ W2)[m-tile] via lhsT = H^T slices (layout flip)
            for m in range(MT):
                pw = psw.tile([128, CODE], f32, space="PSUM")
                nc.tensor.matmul(
                    pw[:, :],
                    lhsT=HT_s[:, m * 128 : (m + 1) * 128],
                    rhs=w2b[:, :],
                    start=True,
                    stop=True,
                )
                sw = wstage.tile([128, CODE], bf16)
                nc.vector.tensor_copy(sw[:, :], pw[:, :])
                nc.sync.dma_start(hw_own[m * 128 : (m + 1) * 128, :], sw[:, :])

            nc.gpsimd.collective_compute(
                "AllGather",
                mybir.AluOpType.bypass,
                replica_groups=groups_all,
                ins=[hw_own.opt()],
                outs=[hw_all.opt()],
            )

            # ---- layer 2: z^T = (A @ Hw)^T  [CODE, R] ----
            htab = load_table(CODE, hw_all)
            ps2 = spmm_T(htab, CODE, pst, False, "l2")
            zT_s = tstage.tile([CODE, R], bf16, tag="zts")
            nc.vector.tensor_copy(zT_s[:, :], ps2[:, :])
            nc.sync.dma_start(zt_own[:, :], zT_s[:, :])
            # own-z decode operand can stage before the AllGather
            for s in range(4):
                nc.vector.tensor_copy(zts4[32 * s : 32 * s + CODE, :], zT_s[:, :])

        nc.gpsimd.collective_compute(
            "AllGather",
            mybir.AluOpType.bypass,
            replica_groups=groups_all,
            ins=[zt_own.opt()],
            outs=[zt_all.opt()],
        )
        # load z^T gathered into 4 partition strips
        for s in range(4):
            nc.sync.dma_start(
                ztall4[32 * s : 32 * s + CODE, :].rearrange(
                    "p (r j) -> p r j", r=cfg.n_cores
                ),
                zt_all.rearrange("r p j -> p r j"),
            )

        # ================= decode =================
        with tc.tile_pool(name="obuf", bufs=4) as obuf, tc.tile_pool(
            name="psd", bufs=2, space="PSUM"
        ) as psd:
            qq = 0
            for m in range(MT):
                rm = min(128, R - m * 128)
                if rm <= 0:
                    continue
                for bg in bank_groups:
                    w = sum(nn for _, nn in bg)
                    pd = psd.tile([128, 2048], f32, space="PSUM")
                    for q, (nn0, nn) in enumerate(bg):
                        s = qq % 4  # rotate PE row strips so LDW pipelines
                        qq += 1
                        p0 = 32 * s
                        nc.tensor.matmul(
                            pd[:rm, q * 512 : q * 512 + nn],
                            lhsT=zts4[p0 : p0 + CODE, m * 128 : m * 128 + rm],
                            rhs=ztall4[p0 : p0 + CODE, nn0 : nn0 + nn],
                            start=True,
                            stop=True,
                            tile_position=(p0, 0),
                        )
                    ob = obuf.tile([128, 2048], bf16)
                    nc.scalar.activation(
                        ob[:rm, :w],
                        pd[:rm, :w],
                        mybir.ActivationFunctionType.Sigmoid,
                    )
                    b0 = bg[0][0]
                    nc.sync.dma_start(
                        out_d[m * 128 : m * 128 + rm, b0 : b0 + w], ob[:rm, :w]
                    )

    nc.compile()
    return nc


def _host_prep(cfg: Cfg, x, W1, W2, edge_weight, src, dst):
    x = np.asarray(x, dtype=np.float32)
    W1 = np.ascontiguousarray(np.asarray(W1, dtype=np.float32))
    W2 = np.ascontiguousarray(np.asarray(W2, dtype=np.float32))
    src = np.asarray(src).astype(np.int64)
    dst = np.asarray(dst).astype(np.int64)
    ew = np.asarray(edge_weight).astype(np.float64)
    R, RP = cfg.rows, cfg.rpad
    # padded node id: n' = 1280*(s//1250) + s%1250
    srcp = RP * (src // R) + (src % R)
    in_maps = []
    for c in range(cfg.n_cores):
        lo = c * R
        m = (dst >= lo) & (dst < lo + R)
        # AB[n', j] = sum of edge weights src -> lo+j, permuted node axis
        flat = srcp[m] * R + (dst[m] - lo)
        D = np.bincount(flat, weights=ew[m], minlength=cfg.npad * R).astype(
            np.float32
        )
        # permuted pair-pack: block k rows are nodes 80p+k ->
        # D[(p k) j] -> [q=k/2, p, l=k%2, j]
        ab = (
            D.reshape(128, cfg.qt, 2, R)
            .transpose(1, 0, 2, 3)
            .reshape(cfg.qt, 128, 2 * R)
            .astype(ml_dtypes.float8_e4m3)
        )
        xpad = np.zeros((RP, cfg.n_feat), np.float32)
        xpad[:R] = x[lo : lo + R]
        in_maps.append(
            {
                "xt": np.ascontiguousarray(xpad.T.astype(ml_dtypes.bfloat16)),
                "w1": W1.astype(ml_dtypes.bfloat16),
                "w2": W2.astype(ml_dtypes.bfloat16),
                "ab": np.ascontiguousarray(ab),
            }
        )
    return in_maps


def kernel(x, W1, W2, edge_weight, src, dst, trace=False):
    cfg = Cfg()
    in_maps = _host_prep(cfg, x, W1, W2, edge_weight, src, dst)
    nc = build_nc(cfg)
    res = run_bass_kernel_spmd(
        nc, in_maps, core_ids=list(range(cfg.n_cores)), trace=trace
    )
    out = np.concatenate([r["out"] for r in res.results], axis=0)
    if trace:
        kernel.last_results = res
    return np.ascontiguousarray(out.astype(np.float32))


# revision 10
# speedup vs baseline: 2.6616x; 1.2173x over previous
"""GCN autoencoder kernel for 8 Trainium2 NeuronCores — dense-block SpMM.

Strategy (self-contained; shapes hardcoded for the graded problem):
  - Nodes row-sharded 1250/core, padded to 1280/core (padded ids
    n' = 1280c + i). Contraction tiles use the permuted layout
    node(p, k) = 80p + k so the gathered feature tables load as one
    contiguous stripe per partition.
  - Host precomputes, per core, the dense adjacency slab
    AB[n', j] = A_hat[base+j, n'] as 40 pair-packed bf16 blocks
    [128, 2, 1250] in the permuted row order — graph-constant layout
    prep. Host also ships x pre-transposed bf16 (zero-padded).
  - Y1 = x @ W1 computed row-wise per m-tile (lhsT = xT slabs).
    AllGather of bf16 Y1 rows [10240, 32].
  - SpMM layers run transposed on PE: out^T[F, 1250] = sum_k T_k^T @ AB_k
    with the table k-tile as stationary weights and the dense AB block
    streamed from HBM as the moving operand (no per-edge DMA gather).
    AB streaming uses the Scalar-engine HWDGE ring so it cannot queue
    ahead of critical-path Sync-ring DMAs. The first 18 pairs stay
    resident in SBUF and are reused by layer 2; only the tail re-streams.
  - relu on ScalarE; Hw = H @ W2 fused with the layout flip back to rows
    (lhsT = H^T m-slices); AllGather Hw rows; layer 2 gives z^T [16, 1250]
    directly; AllGather z^T.
  - Decode: out = sigmoid(z_own @ z_all^T) bf16 matmuls (N=512 chunks,
    4-strip row rotation), ScalarE sigmoid PSUM->SBUF, bf16 output
    stored per 2048-col bank group (host casts back to f32).
"""

from contextlib import ExitStack
from dataclasses import dataclass

import numpy as np
import ml_dtypes

import concourse.bass as bass
import concourse.mybir as mybir
import concourse.tile as tile
from concourse import bacc
from concourse.bass_utils import run_bass_kernel_spmd

dt = mybir.dt


@dataclass
class Cfg:
    n_nodes: int = 10000
    n_feat: int = 512
    hid: int = 32
    code: int = 16
    n_cores: int = 8
    res_pairs: int = 40   # all AB pairs resident in SBUF (fp8), reused by layer 2

    @property
    def rows(self):
        return self.n_nodes // self.n_cores          # 1250 real rows/core

    @property
    def rpad(self):
        return 1280                                   # padded rows/core

    @property
    def npad(self):
        return self.rpad * self.n_cores               # 10240

    @property
    def kt(self):
        return 80                                     # k-tiles (npad/128)

    @property
    def qt(self):
        return 40                                     # AB pairs

    @property
    def mt(self):
        return self.rpad // 128                       # 10 m-tiles/core

    @property
    def kc(self):
        return self.n_feat // 128

    @property
    def rc(self):
        return 1280                                   # padded dst cols (16B lanes)


def _nchunks(total, step=512):
    out = []
    n0 = 0
    while n0 < total:
        out.append((n0, min(step, total - n0)))
        n0 += step
    return out


def build_nc(cfg: Cfg):
    nc = bacc.Bacc(
        "TRN2",
        target_bir_lowering=False,
        debug=False,
        enable_asserts=False,
        num_devices=cfg.n_cores,
    )
    f32 = dt.float32
    bf16 = dt.bfloat16
    N, R, RP, NP = cfg.n_nodes, cfg.rows, cfg.rpad, cfg.npad
    RC = cfg.rc
    HID, CODE = cfg.hid, cfg.code
    KT, QT, MT, KC = cfg.kt, cfg.qt, cfg.mt, cfg.kc

    # ---- external I/O ----
    xT_d = nc.dram_tensor("xt", [cfg.n_feat, RP], bf16, kind="ExternalInput").ap()
    w1_d = nc.dram_tensor("w1", [cfg.n_feat, HID], bf16, kind="ExternalInput").ap()
    w2_d = nc.dram_tensor("w2", [HID, CODE], bf16, kind="ExternalInput").ap()
    ab_d = nc.dram_tensor("ab", [QT, 128, 2 * RC], dt.float8e4, kind="ExternalInput").ap()
    out_d = nc.dram_tensor("out", [R, N], bf16, kind="ExternalOutput").ap()

    # ---- internal DRAM ----
    y1_own = nc.dram_tensor("y1_own", [RP, HID], dt.float8e4).ap()
    y1_all = nc.dram_tensor("y1_all", [NP, HID], dt.float8e4, addr_space="Shared").ap()
    hw_own = nc.dram_tensor("hw_own", [RP, CODE], dt.float8e4).ap()
    hw_all = nc.dram_tensor("hw_all", [NP, CODE], dt.float8e4, addr_space="Shared").ap()
    zt_own = nc.dram_tensor("zt_own", [CODE, R], bf16).ap()
    zt_all = nc.dram_tensor(
        "zt_all", [cfg.n_cores, CODE, R], bf16, addr_space="Shared"
    ).ap()

    dmy_own = nc.dram_tensor("dmy_own", [128], bf16).ap()
    dmy_all = nc.dram_tensor(
        "dmy_all", [128 * cfg.n_cores], bf16, addr_space="Shared"
    ).ap()

    groups_all = [list(range(cfg.n_cores))]
    rchunks = _nchunks(RC)         # psum n-chunking over the padded dst cols

    # decode N-chunking: 512-wide chunks grouped 4 per PSUM tile
    bank_groups = []
    ncs = _nchunks(N)
    for i in range(0, len(ncs), 4):
        bank_groups.append(ncs[i : i + 4])

    with tile.TileContext(nc) as tc, ExitStack() as ctx:
        cpool = ctx.enter_context(tc.tile_pool(name="consts", bufs=1))
        zpool = ctx.enter_context(tc.tile_pool(name="zbits", bufs=1))
        resp = ctx.enter_context(tc.tile_pool(name="abres", bufs=cfg.res_pairs))
        tabp = ctx.enter_context(tc.tile_pool(name="tab", bufs=1))

        w1s = cpool.tile([128, KC, HID], bf16)
        for k in range(KC):
            nc.sync.dma_start(w1s[:, k, :], w1_d[k * 128 : (k + 1) * 128, :])
        w2b = cpool.tile([HID, CODE], bf16)
        nc.sync.dma_start(w2b[:], w2_d[:, :])

        nc.gpsimd.collective_compute(
            "AllGather",
            mybir.AluOpType.bypass,
            replica_groups=groups_all,
            ins=[dmy_own.opt()],
            outs=[dmy_all.opt()],
        )

        # decode operands replicated at 4 partition strips
        zts4 = zpool.tile([128, R], bf16)
        ztall4 = zpool.tile([128, N], bf16)

        def load_table(fdim, src_all):
            """[10240, F] row table -> [128, KT, F]: node 80p+k at (p, k)."""
            tab = tabp.tile([128, KT, max(HID, CODE)], dt.float8e4, tag="tab")
            t = tab[:, :, :fdim]
            nc.sync.dma_start(
                t[:, :, :], src_all.rearrange("(p k) f -> p k f", p=128)
            )
            return t

        # ================= phase Y1: y1 = x @ W1 (row layout) ============
        with tc.tile_pool(name="xts", bufs=1) as xtp, tc.tile_pool(
            name="psy", bufs=4, space="PSUM"
        ) as psy, tc.tile_pool(name="ystage", bufs=4) as ystage:
            xTs = xtp.tile([128, KC, RP], bf16)
            nc.sync.dma_start(
                xTs[:, :, :], xT_d.rearrange("(k p) n -> p k n", p=128)
            )
            for m in range(MT):
                py = psy.tile([128, HID], f32, space="PSUM")
                for k in range(KC):
                    nc.tensor.matmul(
                        py[:, :],
                        lhsT=xTs[:, k, m * 128 : (m + 1) * 128],
                        rhs=w1s[:, k, :],
                        start=(k == 0),
                        stop=(k == KC - 1),
                    )
                st = ystage.tile([128, HID], dt.float8e4)
                nc.vector.tensor_copy(st[:, :], py[:, :])
                nc.sync.dma_start(y1_own[m * 128 : (m + 1) * 128, :], st[:, :])

        nc.gpsimd.collective_compute(
            "AllGather",
            mybir.AluOpType.bypass,
            replica_groups=groups_all,
            ins=[y1_own.opt()],
            outs=[y1_all.opt()],
        )

        # ================= dense SpMM layers =================
        ab_tiles = {}

        def ab_tile(q, layer1):
            if layer1:
                t = resp.tile([128, 2, RC], dt.float8e4, tag="abres")
                nc.scalar.dma_start(
                    t[:, :, :], ab_d[q].rearrange("p (l n) -> p l n", l=2)
                )
                ab_tiles[q] = t
            return ab_tiles[q]

        def spmm_T(tab, fdim, pst, layer1, tag):
            """psum[fdim, RC] = sum_q tabpair_q^T @ ABpair_q (fp8 DoubleRow)."""
            ps = pst.tile([fdim, RC], f32, space="PSUM", tag=f"ps_{tag}")
            for q in range(QT):
                ab = ab_tile(q, layer1)
                for n0, nn in rchunks:
                    nc.tensor.matmul(
                        ps[:, n0 : n0 + nn],
                        lhsT=tab[:, 2 * q : 2 * q + 2, :],
                        rhs=ab[:, :, n0 : n0 + nn],
                        start=(q == 0),
                        stop=(q == QT - 1),
                        perf_mode=mybir.MatmulPerfMode.DoubleRow,
                    )
            return ps

        with tc.tile_pool(name="pst", bufs=1, space="PSUM") as pst, tc.tile_pool(
            name="tstage", bufs=1
        ) as tstage, tc.tile_pool(name="psw", bufs=2, space="PSUM") as psw, tc.tile_pool(
            name="wstage", bufs=4
        ) as wstage:
            # ---- layer 1: H^T = relu(A @ (x W1))^T ----
            ytab = load_table(HID, y1_all)
            ps1 = spmm_T(ytab, HID, pst, True, "l1")
            HT_s = tstage.tile([HID, RP], bf16)
            nc.vector.memset(HT_s[:, R:RP], 0.0)
            nc.scalar.activation(
                HT_s[:, :R], ps1[:, :R], mybir.ActivationFunctionType.Relu
            )
            sgp = tstage.tile([1, 8], bf16, tag="sgp")
            nc.scalar.activation(
                sgp[:, :], w2b[:1, :8], mybir.ActivationFunctionType.Sigmoid
            )
            # Hw rows = (H @ W2)[m-tile] via lhsT = H^T slices (layout flip)
            for m in range(MT):
                pw = psw.tile([128, CODE], f32, space="PSUM")
                nc.tensor.matmul(
                    pw[:, :],
                    lhsT=HT_s[:, m * 128 : (m + 1) * 128],
                    rhs=w2b[:, :],
                    start=True,
                    stop=True,
                )
                sw = wstage.tile([128, CODE], dt.float8e4)
                nc.vector.tensor_copy(sw[:, :], pw[:, :])
                nc.sync.dma_start(hw_own[m * 128 : (m + 1) * 128, :], sw[:, :])

            nc.gpsimd.collective_compute(
                "AllGather",
                mybir.AluOpType.bypass,
                replica_groups=groups_all,
                ins=[hw_own.opt()],
                outs=[hw_all.opt()],
            )

            # ---- layer 2: z^T = (A @ Hw)^T  [CODE, R] ----
            htab = load_table(CODE, hw_all)
            ps2 = spmm_T(htab, CODE, pst, False, "l2")
            zT_s = tstage.tile([CODE, R], bf16, tag="zts")
            nc.vector.tensor_copy(zT_s[:, :], ps2[:, :R])
            nc.sync.dma_start(zt_own[:, :], zT_s[:, :])
            # own-z decode operand can stage before the AllGather
            for s in range(4):
                nc.vector.tensor_copy(zts4[32 * s : 32 * s + CODE, :], zT_s[:, :])

        nc.gpsimd.collective_compute(
            "AllGather",
            mybir.AluOpType.bypass,
            replica_groups=groups_all,
            ins=[zt_own.opt()],
            outs=[zt_all.opt()],
        )
        # load z^T gathered into 4 partition strips
        for s in range(4):
            nc.sync.dma_start(
                ztall4[32 * s : 32 * s + CODE, :].rearrange(
                    "p (r j) -> p r j", r=cfg.n_cores
                ),
                zt_all.rearrange("r p j -> p r j"),
            )

        # ================= decode =================
        with tc.tile_pool(name="obuf", bufs=4) as obuf, tc.tile_pool(
            name="psd", bufs=2, space="PSUM"
        ) as psd:
            qq = 0
            for m in range(MT):
                rm = min(128, R - m * 128)
                if rm <= 0:
                    continue
                for bg in bank_groups:
                    w = sum(nn for _, nn in bg)
                    pd = psd.tile([128, 2048], f32, space="PSUM")
                    for q, (nn0, nn) in enumerate(bg):
                        s = qq % 4  # rotate PE row strips so LDW pipelines
                        qq += 1
                        p0 = 32 * s
                        nc.tensor.matmul(
                            pd[:rm, q * 512 : q * 512 + nn],
                            lhsT=zts4[p0 : p0 + CODE, m * 128 : m * 128 + rm],
                            rhs=ztall4[p0 : p0 + CODE, nn0 : nn0 + nn],
                            start=True,
                            stop=True,
                            tile_position=(p0, 0),
                        )
                    ob = obuf.tile([128, 2048], bf16)
                    nc.scalar.activation(
                        ob[:rm, :w],
                        pd[:rm, :w],
                        mybir.ActivationFunctionType.Sigmoid,
                    )
                    b0 = bg[0][0]
                    nc.sync.dma_start(
                        out_d[m * 128 : m * 128 + rm, b0 : b0 + w], ob[:rm, :w]
                    )

    nc.compile()
    return nc


def _host_prep(cfg: Cfg, x, W1, W2, edge_weight, src, dst):
    x = np.asarray(x, dtype=np.float32)
    W1 = np.ascontiguousarray(np.asarray(W1, dtype=np.float32))
    W2 = np.ascontiguousarray(np.asarray(W2, dtype=np.float32))
    src = np.asarray(src).astype(np.int64)
    dst = np.asarray(dst).astype(np.int64)
    ew = np.asarray(edge_weight).astype(np.float64)
    R, RP = cfg.rows, cfg.rpad
    # padded node id: n' = 1280*(s//1250) + s%1250
    srcp = RP * (src // R) + (src % R)
    in_maps = []
    for c in range(cfg.n_cores):
        lo = c * R
        m = (dst >= lo) & (dst < lo + R)
        # AB[n', j] = sum of edge weights src -> lo+j, permuted node axis,
        # dst cols padded to RC for 16B DoubleRow lane alignment
        RC = cfg.rc
        flat = srcp[m] * RC + (dst[m] - lo)
        D = np.bincount(flat, weights=ew[m], minlength=cfg.npad * RC).astype(
            np.float32
        )
        # permuted pair-pack: block k rows are nodes 80p+k ->
        # D[(p k) j] -> [q=k/2, p, l=k%2, j]
        ab = (
            D.reshape(128, cfg.qt, 2, RC)
            .transpose(1, 0, 2, 3)
            .reshape(cfg.qt, 128, 2 * RC)
            .astype(ml_dtypes.float8_e4m3)
        )
        xpad = np.zeros((RP, cfg.n_feat), np.float32)
        xpad[:R] = x[lo : lo + R]
        in_maps.append(
            {
                "xt": np.ascontiguousarray(xpad.T.astype(ml_dtypes.bfloat16)),
                "w1": W1.astype(ml_dtypes.bfloat16),
                "w2": W2.astype(ml_dtypes.bfloat16),
                "ab": np.ascontiguousarray(ab),
            }
        )
    return in_maps


def kernel(x, W1, W2, edge_weight, src, dst, trace=False):
    cfg = Cfg()
    in_maps = _host_prep(cfg, x, W1, W2, edge_weight, src, dst)
    nc = build_nc(cfg)
    res = run_bass_kernel_spmd(
        nc, in_maps, core_ids=list(range(cfg.n_cores)), trace=trace
    )
    out = np.concatenate([r["out"] for r in res.results], axis=0)
    if trace:
        kernel.last_results = res
    return np.ascontiguousarray(out.astype(np.float32))
